# revision 16
# baseline (speedup 1.0000x reference)
"""Causal self-attention (B=4, T=2048, C=1024, H=16) on 8 Trainium2 cores.

Sharding: tensor-parallel over heads. Core c owns heads (2c, 2c+1):
  - QKV projection in bf16: Q^T/K^T computed in [dim, token] layout with the
    host-pretransposed x^T (bf16) as the moving operand; V computed directly
    in [token, dim] layout (x^T t-block stationary, bf16 W_v moving, N=128)
    so no PE transposes are needed.
  - Attention in the S^T = [j, i] orientation so softmax'd probabilities
    feed the AV matmul untransposed. AV emission runs two j-steps behind
    QK, and the next chunk's QKV-projection matmuls are pumped into the
    attention loop in small quanta, so the PE always has independent work
    while the exp (ACT) of earlier steps completes.
  - Softmax denominator rides the AV matmul (augmented-V ones column); the
    two heads' reciprocal rows are broadcast across partitions by GPSIMD
    partition_broadcast (all custom ops at base partition 0).
  - Output projection in f32r: per-core partial [BT, C]; partials summed on
    host (row-parallel tensor parallelism). Normalize + proj + store are
    pipelined per 128-token tile to shorten the tail.
"""

import numpy as np

N_CORES = 8
B, T, C = 4, 2048, 1024
H, Dh = 16, 64
BT = B * T  # 8192
TCH = 512  # t-chunk (stage 1) and i-chunk (stage 2)

_CACHE = {}


def _build():
    import concourse.bacc as bacc
    import concourse.mybir as mybir
    import concourse.tile as tile
    from contextlib import ExitStack

    f32 = mybir.dt.float32
    f32r = mybir.dt.float32r
    bf16 = mybir.dt.bfloat16
    Exp = mybir.ActivationFunctionType.Exp
    add = mybir.AluOpType.add
    mult = mybir.AluOpType.mult
    is_ge = mybir.AluOpType.is_ge

    nc = bacc.Bacc(None, target_bir_lowering=False, debug=False)
    x_t = nc.dram_tensor("x_t", [C, BT], bf16, kind="ExternalInput").ap()
    w_qk = nc.dram_tensor("w_qk", [C, 256], bf16, kind="ExternalInput").ap()
    w_v = nc.dram_tensor("w_v", [C, 128], bf16, kind="ExternalInput").ap()
    b_qk = nc.dram_tensor("b_qk", [128, 2], f32, kind="ExternalInput").ap()
    b_v = nc.dram_tensor("b_v", [128, 128], f32, kind="ExternalInput").ap()
    w_proj = nc.dram_tensor("w_proj", [128, C], f32r, kind="ExternalInput").ap()
    out = nc.dram_tensor("out", [BT, C], f32, kind="ExternalOutput").ap()

    scale = 1.0 / float(np.sqrt(Dh))

    with tile.TileContext(nc) as tc, ExitStack() as ctx:
        persist = ctx.enter_context(tc.tile_pool(name="persist", bufs=1))
        xt_pool = ctx.enter_context(tc.tile_pool(name="xt", bufs=2))
        p_pool = ctx.enter_context(tc.tile_pool(name="pp", bufs=8))
        rc_pool = ctx.enter_context(tc.tile_pool(name="rc", bufs=3))
        yt_pool = ctx.enter_context(tc.tile_pool(name="yt", bufs=2))
        ob_pool = ctx.enter_context(tc.tile_pool(name="ob", bufs=3))
        # PSUM 8 banks: s_ps 2x[128,1024]=4, psA/psB 2x[128,512]=2, work 2
        ps_big = ctx.enter_context(tc.tile_pool(name="psb", bufs=2, space="PSUM"))
        ps_y = ctx.enter_context(tc.tile_pool(name="psy", bufs=2, space="PSUM"))
        ps_work = ctx.enter_context(tc.tile_pool(name="psw", bufs=2, space="PSUM"))

        QT = persist.tile([128, BT], bf16, tag="QT")
        KT = persist.tile([128, BT], bf16, tag="KT")
        # augmented V, bf16: per global t-tile gt a [128, 128] stationary block
        #   VnA[:, gt, 0:64] = V_A, [.., 64] = 1.0  (65: never read from psA)
        #   VnB[:, gt, 0] = 1.0, [.., 64:128] = V_B (1:64 never read from psB)
        VnA = persist.tile([128, 64, 128], bf16, tag="VnA")
        VnB = persist.tile([128, 64, 128], bf16, tag="VnB")
        wqk_sb = persist.tile([128, 8, 256], bf16, tag="wqk")
        wv_sb = persist.tile([128, 8, 128], bf16, tag="wv")
        wp_sb = persist.tile([128, C], f32r, tag="wp")
        bqk_sb = persist.tile([128, 2], f32, tag="bqk")
        bv_sb = persist.tile([128, 128], f32, tag="bv")

        # weights via SWDGE ring (Pool queue), cc0 of W_qk first so the very
        # first projection matmul can start while the rest streams in.
        wqk_src = w_qk.rearrange("(cc p) j -> p cc j", p=128)
        nc.gpsimd.dma_start(wqk_sb[:, 0, :], wqk_src[:, 0, :])
        nc.gpsimd.dma_start(wqk_sb[:, 1:8, :], wqk_src[:, 1:8, :])
        nc.gpsimd.dma_start(wv_sb[:], w_v.rearrange("(cc p) j -> p cc j", p=128))
        nc.gpsimd.dma_start(bqk_sb[:], b_qk[:])
        nc.gpsimd.dma_start(bv_sb[:], b_v[:])
        # only the ones columns matter: V columns are written by stage 1 and
        # the remaining columns feed PSUM partitions that are never read.
        nc.gpsimd.memset(VnA[:, :, 64:65], 1.0)
        nc.gpsimd.memset(VnB[:, :, 0:1], 1.0)
        nc.gpsimd.dma_start(wp_sb[:], w_proj[:])

        def stage1_quanta(tci):
            """QKV for t-chunk tci as a list of emission quanta."""
            xt = xt_pool.tile([128, 8, TCH], bf16, tag="xt")
            quanta = []

            def dma_in():
                src = x_t[:, tci * TCH : (tci + 1) * TCH].rearrange(
                    "(cc p) t -> p cc t", p=128
                )
                if tci == 0:
                    for cc in range(8):
                        nc.sync.dma_start(xt[:, cc, :], src[:, cc, :])
                else:
                    nc.sync.dma_start(xt[:], src)

            quanta.append(dma_in)
            tsl = slice(tci * TCH, (tci + 1) * TCH)
            ps_box = {}

            def qk_mms(jt, cc0):
                def f():
                    if cc0 == 0:
                        ps_box[jt] = ps_work.tile([128, TCH], f32, tag="psw", name="s1ps")
                    ps = ps_box[jt]
                    for cc in (cc0, cc0 + 1, cc0 + 2, cc0 + 3):
                        nc.tensor.matmul(
                            ps[:],
                            lhsT=wqk_sb[:, cc, jt * 128 : (jt + 1) * 128],
                            rhs=xt[:, cc, :],
                            start=(cc == 0),
                            stop=(cc == 7),
                        )

                return f

            def qk_evict(jt):
                def f():
                    dest = QT if jt == 0 else KT
                    nc.vector.tensor_scalar(
                        dest[:, tsl], ps_box[jt][:], bqk_sb[:, jt : jt + 1], None, add
                    )

                return f

            for jt in range(2):
                quanta.append(qk_mms(jt, 0))
                quanta.append(qk_mms(jt, 4))
                quanta.append(qk_evict(jt))

            vp_box = {}

            def v_mms(q):
                def f():
                    if q == 0:
                        vp_box[0] = ps_work.tile([128, TCH], f32, tag="psw", name="s1vp")
                    vp = vp_box[0]
                    for cc in range(8):
                        nc.tensor.matmul(
                            vp[:, q * 128 : (q + 1) * 128],
                            lhsT=xt[:, cc, q * 128 : (q + 1) * 128],
                            rhs=wv_sb[:, cc, :],
                            start=(cc == 0),
                            stop=(cc == 7),
                            skip_group_check=True,
                        )

                return f

            def v_evict(q):
                def f():
                    gt = tci * 4 + q
                    vp = vp_box[0]
                    nc.vector.tensor_tensor(
                        out=VnA[:, gt, 0:64],
                        in0=vp[:, q * 128 : q * 128 + 64],
                        in1=bv_sb[:, 0:64],
                        op=add,
                    )
                    nc.vector.tensor_tensor(
                        out=VnB[:, gt, 64:128],
                        in0=vp[:, q * 128 + 64 : (q + 1) * 128],
                        in1=bv_sb[:, 64:128],
                        op=add,
                    )

                return f

            for q in range(4):
                quanta.append(v_mms(q))
                quanta.append(v_evict(q))

            return quanta

        def attention_step(b, ic, yT, pump):
            """S^T attention for i-chunk ic of batch b -> normalized yT cols,
            proj + store, pumping stage-1 quanta of the next chunk into PE
            bubbles. `pump(n)` emits up to n pending quanta."""
            jt_n = 4 * ic + 4
            icol = b * T + ic * TCH
            psA = ps_y.tile([128, TCH], f32, tag="psy")  # yA 0:64, denA row 64
            psB = ps_y.tile([128, TCH], f32, tag="psy")  # denB row 0, yB 64:128
            p_tiles = [None] * jt_n
            col0s = [None] * jt_n

            def emit_av(jt):
                p, col0 = p_tiles[jt], col0s[jt]
                gt = b * 16 + jt
                first, last = (jt == 0), (jt == jt_n - 1)
                nc.tensor.matmul(
                    psA[:, col0:512],
                    lhsT=VnA[:, gt, :],
                    rhs=p[:, 0, col0:],
                    start=first,
                    stop=last,
                    skip_group_check=True,
                )
                nc.tensor.matmul(
                    psB[:, col0:512],
                    lhsT=VnB[:, gt, :],
                    rhs=p[:, 1, col0:],
                    start=first,
                    stop=last,
                    skip_group_check=True,
                )

            for jt in range(jt_n):
                d = jt - 4 * ic
                col0 = 128 * d if d >= 0 else 0
                col0s[jt] = col0
                jcol = b * T + jt * 128
                s_ps = ps_big.tile([128, 1024], f32, tag="psb")
                # QK^T row-packed: head A rows 0:64, head B rows 64:128
                nc.tensor.matmul(
                    s_ps[:, col0:512],
                    lhsT=KT[0:64, jcol : jcol + 128],
                    rhs=QT[0:64, icol + col0 : icol + 512],
                    start=True,
                    stop=True,
                )
                nc.tensor.matmul(
                    s_ps[:, 512 + col0 : 1024],
                    lhsT=KT[64:128, jcol : jcol + 128],
                    rhs=QT[64:128, icol + col0 : icol + 512],
                    start=True,
                    stop=True,
                )
                p = p_pool.tile([128, 2, TCH], bf16, tag="pp")
                p_tiles[jt] = p
                s3 = s_ps[:].rearrange("p (h n) -> p h n", h=2)
                nc.scalar.activation(p[:, :, col0:], s3[:, :, col0:], Exp, scale=scale)
                if d >= 0:
                    # zero the upper triangle of the diagonal 128x128 block
                    nc.gpsimd.affine_select(
                        out=p[:, :, col0 : col0 + 128],
                        in_=p[:, :, col0 : col0 + 128],
                        pattern=[[0, 2], [1, 128]],
                        compare_op=is_ge,
                        fill=0.0,
                        base=0,
                        channel_multiplier=-1,
                    )
                # software pipeline: AV two j-steps behind QK/exp
                if jt >= 2:
                    emit_av(jt - 2)
                pump(1)
            if jt_n >= 2:
                emit_av(jt_n - 2)
            pump(2)
            emit_av(jt_n - 1)

            # denominators: stage denA (psA row 64) to partition 0, take both
            # reciprocals side by side on partition 0, broadcast across
            # partitions on GPSIMD.
            dn = rc_pool.tile([128, 2 * TCH], f32, tag="dn")
            nc.vector.tensor_copy(out=dn[0:1, 0:TCH], in_=psA[64:65, :])
            nc.vector.reciprocal_approx_fast(dn[0:1, TCH : 2 * TCH], psB[0:1, :])
            nc.vector.reciprocal_approx_fast(dn[0:1, 0:TCH], dn[0:1, 0:TCH])
            rcA = rc_pool.tile([128, TCH], f32, tag="rcA")
            rcB = rc_pool.tile([128, TCH], f32, tag="rcB")
            nc.gpsimd.partition_broadcast(rcA[:, :], dn[0:1, 0:TCH])
            nc.gpsimd.partition_broadcast(rcB[:, :], dn[0:1, TCH : 2 * TCH])
            pump(2)

            # normalize per 128-token tile (releases psA/psB), then project +
            # store; o_ps reuses the ps_y pool slots psA/psB vacate.
            for q in range(TCH // 128):
                lt = ic * TCH + q * 128  # col in yT
                cs = slice(q * 128, (q + 1) * 128)
                nc.vector.tensor_tensor(
                    out=yT[0:64, lt : lt + 128], in0=psA[0:64, cs], in1=rcA[0:64, cs],
                    op=mult,
                )
                nc.vector.tensor_tensor(
                    out=yT[64:128, lt : lt + 128], in0=psB[64:128, cs],
                    in1=rcB[64:128, cs], op=mult,
                )
            pump(1)
            for q in range(TCH // 128):
                lt = ic * TCH + q * 128
                tt = (b * T + ic * TCH) // 128 + q  # global out row-tile
                ob = ob_pool.tile([128, C], f32, tag="ob")
                for nh in range(2):
                    o_ps = ps_y.tile([128, 512], f32, tag="psy")
                    nc.tensor.matmul(
                        o_ps[:],
                        lhsT=yT[:, lt : lt + 128],
                        rhs=wp_sb[:, nh * 512 : (nh + 1) * 512],
                        start=True,
                        stop=True,
                    )
                    nc.vector.tensor_copy(
                        out=ob[:, nh * 512 : (nh + 1) * 512], in_=o_ps[:]
                    )
                nc.sync.dma_start(out[tt * 128 : (tt + 1) * 128, :], ob[:])
                pump(2)

        pending = []

        def pump(n):
            for _ in range(min(n, len(pending))):
                pending.pop(0)()

        for q in stage1_quanta(0):
            q()
        for step in range(16):
            b, ic = divmod(step, 4)
            if ic == 0:
                yT = yt_pool.tile([128, T], f32r, tag="yt")
            if step + 1 < 16:
                pending.extend(stage1_quanta(step + 1))
                pending.pop(0)()  # issue the DMA prefetch immediately
            attention_step(b, ic, yT, pump)
            pump(len(pending))  # flush any leftovers before the next step

    nc.compile()
    return nc


def _get_nc():
    if "nc" not in _CACHE:
        _CACHE["nc"] = _build()
    return _CACHE["nc"]


def _run(inputs, trace=False):
    import ml_dtypes
    from concourse import bass_utils

    bfloat16 = ml_dtypes.bfloat16

    x = np.asarray(inputs["x"], dtype=np.float32)
    W_attn = np.asarray(inputs["W_attn"], dtype=np.float32)
    b_attn = np.asarray(inputs["b_attn"], dtype=np.float32)
    W_proj = np.asarray(inputs["W_proj"], dtype=np.float32)
    b_proj = np.asarray(inputs["b_proj"], dtype=np.float32)

    xT = np.ascontiguousarray(x.reshape(BT, C).T).astype(bfloat16)  # [C, BT]

    in_maps = []
    for c in range(N_CORES):
        hA, hB = 2 * c, 2 * c + 1
        qk_cols = []
        for part in range(2):  # q, k
            base = part * C
            qk_cols.extend(range(base + hA * Dh, base + hA * Dh + Dh))
            qk_cols.extend(range(base + hB * Dh, base + hB * Dh + Dh))
        qk_cols = np.array(qk_cols)
        v_cols = np.array(
            list(range(2 * C + hA * Dh, 2 * C + hA * Dh + Dh))
            + list(range(2 * C + hB * Dh, 2 * C + hB * Dh + Dh))
        )
        in_maps.append(
            {
                "x_t": xT,
                "w_qk": np.ascontiguousarray(W_attn[:, qk_cols]).astype(bfloat16),
                "w_v": np.ascontiguousarray(W_attn[:, v_cols]).astype(bfloat16),
                "b_qk": np.ascontiguousarray(b_attn[qk_cols].reshape(2, 128).T),
                "b_v": np.ascontiguousarray(
                    np.broadcast_to(b_attn[v_cols], (128, 128))
                ),
                "w_proj": np.ascontiguousarray(W_proj[c * 128 : (c + 1) * 128, :]),
            }
        )

    nc = _get_nc()
    res = bass_utils.run_bass_kernel_spmd(
        nc, in_maps, core_ids=list(range(N_CORES)), trace=trace
    )
    acc = res.results[0]["out"].astype(np.float64)
    for c in range(1, N_CORES):
        acc += res.results[c]["out"]
    acc += b_proj
    return acc.reshape(B, T, C).astype(np.float32), res


def kernel(**inputs):
    out, _ = _run(inputs, trace=False)
    return out


def kernel_traced(**inputs):
    _, res = _run(inputs, trace=True)
    return res


# revision 19
# speedup vs baseline: 1.0044x; 1.0044x over previous
"""Causal self-attention (B=4, T=2048, C=1024, H=16) on 8 Trainium2 cores.

Sharding: tensor-parallel over heads. Core c owns heads (2c, 2c+1):
  - QKV projection in bf16: Q^T/K^T computed in [dim, token] layout with the
    host-pretransposed x^T (bf16) as the moving operand; V computed directly
    in [token, dim] layout (x^T t-block stationary, bf16 W_v moving, N=128)
    so no PE transposes are needed.
  - Attention in the S^T = [j, i] orientation so softmax'd probabilities
    feed the AV matmul untransposed. AV emission runs two j-steps behind
    QK, and the next chunk's QKV-projection matmuls are pumped into the
    attention loop in small quanta, so the PE always has independent work
    while the exp (ACT) of earlier steps completes.
  - Softmax denominator rides the AV matmul (augmented-V ones column); the
    two heads' reciprocal rows are broadcast across partitions by GPSIMD
    partition_broadcast (all custom ops at base partition 0).
  - Output projection in f32r: per-core partial [BT, C]; partials summed on
    host (row-parallel tensor parallelism). Normalize + proj + store are
    pipelined per 128-token tile to shorten the tail.
"""

import numpy as np

N_CORES = 8
B, T, C = 4, 2048, 1024
H, Dh = 16, 64
BT = B * T  # 8192
TCH = 512  # t-chunk (stage 1) and i-chunk (stage 2)

_CACHE = {}


def _build():
    import concourse.bacc as bacc
    import concourse.mybir as mybir
    import concourse.tile as tile
    from contextlib import ExitStack

    f32 = mybir.dt.float32
    f32r = mybir.dt.float32r
    bf16 = mybir.dt.bfloat16
    Exp = mybir.ActivationFunctionType.Exp
    Copy = mybir.ActivationFunctionType.Copy
    add = mybir.AluOpType.add
    mult = mybir.AluOpType.mult
    is_ge = mybir.AluOpType.is_ge

    nc = bacc.Bacc(None, target_bir_lowering=False, debug=False)
    x_t = nc.dram_tensor("x_t", [C, BT], bf16, kind="ExternalInput").ap()
    w_qk = nc.dram_tensor("w_qk", [C, 256], bf16, kind="ExternalInput").ap()
    w_v = nc.dram_tensor("w_v", [C, 128], bf16, kind="ExternalInput").ap()
    b_qk = nc.dram_tensor("b_qk", [128, 2], f32, kind="ExternalInput").ap()
    b_v = nc.dram_tensor("b_v", [128, 128], f32, kind="ExternalInput").ap()
    w_proj = nc.dram_tensor("w_proj", [128, C], f32r, kind="ExternalInput").ap()
    out = nc.dram_tensor("out", [BT, C], f32, kind="ExternalOutput").ap()

    scale = 1.0 / float(np.sqrt(Dh))

    with tile.TileContext(nc) as tc, ExitStack() as ctx:
        persist = ctx.enter_context(tc.tile_pool(name="persist", bufs=1))
        xt_pool = ctx.enter_context(tc.tile_pool(name="xt", bufs=3))
        p_pool = ctx.enter_context(tc.tile_pool(name="pp", bufs=8))
        rc_pool = ctx.enter_context(tc.tile_pool(name="rc", bufs=3))
        yt_pool = ctx.enter_context(tc.tile_pool(name="yt", bufs=2))
        ob_pool = ctx.enter_context(tc.tile_pool(name="ob", bufs=3))
        # PSUM 8 banks: s_ps 2x[128,1024]=4, psA/psB 2x[128,512]=2, work 2
        ps_big = ctx.enter_context(tc.tile_pool(name="psb", bufs=2, space="PSUM"))
        ps_y = ctx.enter_context(tc.tile_pool(name="psy", bufs=2, space="PSUM"))
        ps_work = ctx.enter_context(tc.tile_pool(name="psw", bufs=2, space="PSUM"))

        QT = persist.tile([128, BT], bf16, tag="QT")
        KT = persist.tile([128, BT], bf16, tag="KT")
        # augmented V, bf16: per global t-tile gt a [128, 128] stationary block
        #   VnA[:, gt, 0:64] = V_A, [.., 64] = 1.0  (65: never read from psA)
        #   VnB[:, gt, 0] = 1.0, [.., 64:128] = V_B (1:64 never read from psB)
        VnA = persist.tile([128, 64, 128], bf16, tag="VnA")
        VnB = persist.tile([128, 64, 128], bf16, tag="VnB")
        wqk_sb = persist.tile([128, 8, 256], bf16, tag="wqk")
        wv_sb = persist.tile([128, 8, 128], bf16, tag="wv")
        wp_sb = persist.tile([128, C], f32r, tag="wp")
        bqk_sb = persist.tile([128, 2], f32, tag="bqk")
        bv_sb = persist.tile([128, 128], f32, tag="bv")

        # weights via SWDGE ring (Pool queue), cc0 of W_qk first so the very
        # first projection matmul can start while the rest streams in.
        wqk_src = w_qk.rearrange("(cc p) j -> p cc j", p=128)
        nc.sync.dma_start(wqk_sb[:, 0, :], wqk_src[:, 0, :])
        nc.gpsimd.dma_start(wqk_sb[:, 1:8, :], wqk_src[:, 1:8, :])
        nc.gpsimd.dma_start(wv_sb[:], w_v.rearrange("(cc p) j -> p cc j", p=128))
        nc.gpsimd.dma_start(bqk_sb[:], b_qk[:])
        nc.gpsimd.dma_start(bv_sb[:], b_v[:])
        # only the ones columns matter: V columns are written by stage 1 and
        # the remaining columns feed PSUM partitions that are never read.
        nc.gpsimd.memset(VnA[:, :, 64:65], 1.0)
        nc.gpsimd.memset(VnB[:, :, 0:1], 1.0)
        nc.gpsimd.dma_start(wp_sb[:], w_proj[:])

        def stage1_quanta(tci):
            """QKV for t-chunk tci as a list of emission quanta."""
            xt = xt_pool.tile([128, 8, TCH], bf16, tag="xt")
            quanta = []

            def dma_in():
                src = x_t[:, tci * TCH : (tci + 1) * TCH].rearrange(
                    "(cc p) t -> p cc t", p=128
                )
                if tci == 0:
                    for cc in range(8):
                        nc.sync.dma_start(xt[:, cc, :], src[:, cc, :])
                else:
                    nc.sync.dma_start(xt[:], src)

            dma_quantum = dma_in
            tsl = slice(tci * TCH, (tci + 1) * TCH)
            ps_box = {}

            def qk_mms(jt, cc0):
                def f():
                    if cc0 == 0:
                        ps_box[jt] = ps_work.tile([128, TCH], f32, tag="psw", name="s1ps")
                    ps = ps_box[jt]
                    for cc in (cc0, cc0 + 1, cc0 + 2, cc0 + 3):
                        nc.tensor.matmul(
                            ps[:],
                            lhsT=wqk_sb[:, cc, jt * 128 : (jt + 1) * 128],
                            rhs=xt[:, cc, :],
                            start=(cc == 0),
                            stop=(cc == 7),
                        )

                return f

            def qk_evict(jt):
                def f():
                    dest = QT if jt == 0 else KT
                    nc.vector.tensor_scalar(
                        dest[:, tsl], ps_box[jt][:], bqk_sb[:, jt : jt + 1], None, add
                    )

                return f

            for jt in range(2):
                quanta.append(qk_mms(jt, 0))
                quanta.append(qk_mms(jt, 4))
                quanta.append(qk_evict(jt))

            vp_box = {}

            def v_mms(q):
                def f():
                    if q == 0:
                        vp_box[0] = ps_work.tile([128, TCH], f32, tag="psw", name="s1vp")
                    vp = vp_box[0]
                    for cc in range(8):
                        nc.tensor.matmul(
                            vp[:, q * 128 : (q + 1) * 128],
                            lhsT=xt[:, cc, q * 128 : (q + 1) * 128],
                            rhs=wv_sb[:, cc, :],
                            start=(cc == 0),
                            stop=(cc == 7),
                            skip_group_check=True,
                        )

                return f

            def v_evict(q):
                def f():
                    gt = tci * 4 + q
                    vp = vp_box[0]
                    nc.vector.tensor_tensor(
                        out=VnA[:, gt, 0:64],
                        in0=vp[:, q * 128 : q * 128 + 64],
                        in1=bv_sb[:, 0:64],
                        op=add,
                    )
                    nc.vector.tensor_tensor(
                        out=VnB[:, gt, 64:128],
                        in0=vp[:, q * 128 + 64 : (q + 1) * 128],
                        in1=bv_sb[:, 64:128],
                        op=add,
                    )

                return f

            for q in range(4):
                quanta.append(v_mms(q))
                quanta.append(v_evict(q))

            return dma_quantum, quanta

        def attention_step(b, ic, yT, pump):
            """S^T attention for i-chunk ic of batch b -> normalized yT cols,
            proj + store, pumping stage-1 quanta of the next chunk into PE
            bubbles. `pump(n)` emits up to n pending quanta."""
            jt_n = 4 * ic + 4
            icol = b * T + ic * TCH
            psA = ps_y.tile([128, TCH], f32, tag="psy")  # yA 0:64, denA row 64
            psB = ps_y.tile([128, TCH], f32, tag="psy")  # denB row 0, yB 64:128
            p_tiles = [None] * jt_n
            col0s = [None] * jt_n

            def emit_av(jt):
                p, col0 = p_tiles[jt], col0s[jt]
                gt = b * 16 + jt
                first, last = (jt == 0), (jt == jt_n - 1)
                nc.tensor.matmul(
                    psA[:, col0:512],
                    lhsT=VnA[:, gt, :],
                    rhs=p[:, 0, col0:],
                    start=first,
                    stop=last,
                    skip_group_check=True,
                )
                nc.tensor.matmul(
                    psB[:, col0:512],
                    lhsT=VnB[:, gt, :],
                    rhs=p[:, 1, col0:],
                    start=first,
                    stop=last,
                    skip_group_check=True,
                )

            for jt in range(jt_n):
                d = jt - 4 * ic
                col0 = 128 * d if d >= 0 else 0
                col0s[jt] = col0
                jcol = b * T + jt * 128
                s_ps = ps_big.tile([128, 1024], f32, tag="psb")
                # QK^T row-packed: head A rows 0:64, head B rows 64:128
                nc.tensor.matmul(
                    s_ps[:, col0:512],
                    lhsT=KT[0:64, jcol : jcol + 128],
                    rhs=QT[0:64, icol + col0 : icol + 512],
                    start=True,
                    stop=True,
                )
                nc.tensor.matmul(
                    s_ps[:, 512 + col0 : 1024],
                    lhsT=KT[64:128, jcol : jcol + 128],
                    rhs=QT[64:128, icol + col0 : icol + 512],
                    start=True,
                    stop=True,
                )
                p = p_pool.tile([128, 2, TCH], bf16, tag="pp")
                p_tiles[jt] = p
                s3 = s_ps[:].rearrange("p (h n) -> p h n", h=2)
                nc.scalar.activation(p[:, :, col0:], s3[:, :, col0:], Exp, scale=scale)
                if d >= 0:
                    # zero the upper triangle of the diagonal 128x128 block
                    nc.gpsimd.affine_select(
                        out=p[:, :, col0 : col0 + 128],
                        in_=p[:, :, col0 : col0 + 128],
                        pattern=[[0, 2], [1, 128]],
                        compare_op=is_ge,
                        fill=0.0,
                        base=0,
                        channel_multiplier=-1,
                    )
                # software pipeline: AV two j-steps behind QK/exp
                if jt == 0:
                    pump(2)
                if jt >= 2:
                    emit_av(jt - 2)
                pump(1)
            if jt_n >= 2:
                emit_av(jt_n - 2)
            pump(2)
            emit_av(jt_n - 1)

            # denominators in two column halves, pipelined into normalize +
            # project + store per 128-token tile: stage denA (psA row 64) to
            # partition 0, reciprocals on partition 0, GPSIMD broadcast.
            HH = TCH // 2
            dn = rc_pool.tile([128, 2 * TCH], f32, tag="dn")
            rcA = rc_pool.tile([128, TCH], f32, tag="rcA")
            rcB = rc_pool.tile([128, TCH], f32, tag="rcB")
            for h in range(2):
                hs = slice(h * HH, (h + 1) * HH)
                nc.vector.tensor_copy(out=dn[0:1, h * HH : (h + 1) * HH],
                                      in_=psA[64:65, hs])
                nc.vector.reciprocal_approx_fast(
                    dn[0:1, TCH + h * HH : TCH + (h + 1) * HH], psB[0:1, hs]
                )
                nc.vector.reciprocal_approx_fast(
                    dn[0:1, h * HH : (h + 1) * HH], dn[0:1, h * HH : (h + 1) * HH]
                )
                nc.gpsimd.partition_broadcast(rcA[:, hs], dn[0:1, h * HH : (h + 1) * HH])
                nc.gpsimd.partition_broadcast(
                    rcB[:, hs], dn[0:1, TCH + h * HH : TCH + (h + 1) * HH]
                )
                for q in (2 * h, 2 * h + 1):
                    lt = ic * TCH + q * 128  # col in yT
                    cs = slice(q * 128, (q + 1) * 128)
                    nc.vector.tensor_tensor(
                        out=yT[0:64, lt : lt + 128], in0=psA[0:64, cs],
                        in1=rcA[0:64, cs], op=mult,
                    )
                    nc.vector.tensor_tensor(
                        out=yT[64:128, lt : lt + 128], in0=psB[64:128, cs],
                        in1=rcB[64:128, cs], op=mult,
                    )
                    pump(1)
            for q in range(TCH // 128):
                lt = ic * TCH + q * 128
                tt = (b * T + ic * TCH) // 128 + q  # global out row-tile
                ob = ob_pool.tile([128, C], f32, tag="ob")
                for nh in range(2):
                    o_ps = ps_y.tile([128, 512], f32, tag="psy")
                    nc.tensor.matmul(
                        o_ps[:],
                        lhsT=yT[:, lt : lt + 128],
                        rhs=wp_sb[:, nh * 512 : (nh + 1) * 512],
                        start=True,
                        stop=True,
                    )
                    # split evictions across ACT and DVE so neither serializes
                    if nh == 0:
                        nc.scalar.activation(ob[:, 0:512], o_ps[:], Copy)
                    else:
                        nc.vector.tensor_copy(out=ob[:, 512:1024], in_=o_ps[:])
                nc.sync.dma_start(out[tt * 128 : (tt + 1) * 128, :], ob[:])
                pump(2)

        pending = []

        def pump(n):
            for _ in range(min(n, len(pending))):
                pending.pop(0)()

        dma0, rest0 = stage1_quanta(0)
        dma0()
        for q in rest0:
            q()
        dma1, rest1 = stage1_quanta(1)
        dma1()
        rest_next = rest1
        for step in range(16):
            b, ic = divmod(step, 4)
            if ic == 0:
                yT = yt_pool.tile([128, T], f32r, tag="yt")
            if rest_next is not None:
                pending.extend(rest_next)
                rest_next = None
            if step + 2 < 16:
                # issue the x DMA two steps ahead; its compute quanta are
                # pumped during the NEXT step
                dma_n, rest_next = stage1_quanta(step + 2)
                dma_n()
            attention_step(b, ic, yT, pump)
            pump(len(pending))  # flush any leftovers before the next step

    nc.compile()
    return nc


def _get_nc():
    if "nc" not in _CACHE:
        _CACHE["nc"] = _build()
    return _CACHE["nc"]


def _run(inputs, trace=False):
    import ml_dtypes
    from concourse import bass_utils

    bfloat16 = ml_dtypes.bfloat16

    x = np.asarray(inputs["x"], dtype=np.float32)
    W_attn = np.asarray(inputs["W_attn"], dtype=np.float32)
    b_attn = np.asarray(inputs["b_attn"], dtype=np.float32)
    W_proj = np.asarray(inputs["W_proj"], dtype=np.float32)
    b_proj = np.asarray(inputs["b_proj"], dtype=np.float32)

    xT = np.ascontiguousarray(x.reshape(BT, C).T).astype(bfloat16)  # [C, BT]

    in_maps = []
    for c in range(N_CORES):
        hA, hB = 2 * c, 2 * c + 1
        qk_cols = []
        for part in range(2):  # q, k
            base = part * C
            qk_cols.extend(range(base + hA * Dh, base + hA * Dh + Dh))
            qk_cols.extend(range(base + hB * Dh, base + hB * Dh + Dh))
        qk_cols = np.array(qk_cols)
        v_cols = np.array(
            list(range(2 * C + hA * Dh, 2 * C + hA * Dh + Dh))
            + list(range(2 * C + hB * Dh, 2 * C + hB * Dh + Dh))
        )
        in_maps.append(
            {
                "x_t": xT,
                "w_qk": np.ascontiguousarray(W_attn[:, qk_cols]).astype(bfloat16),
                "w_v": np.ascontiguousarray(W_attn[:, v_cols]).astype(bfloat16),
                "b_qk": np.ascontiguousarray(b_attn[qk_cols].reshape(2, 128).T),
                "b_v": np.ascontiguousarray(
                    np.broadcast_to(b_attn[v_cols], (128, 128))
                ),
                "w_proj": np.ascontiguousarray(W_proj[c * 128 : (c + 1) * 128, :]),
            }
        )

    nc = _get_nc()
    res = bass_utils.run_bass_kernel_spmd(
        nc, in_maps, core_ids=list(range(N_CORES)), trace=trace
    )
    acc = res.results[0]["out"].astype(np.float64)
    for c in range(1, N_CORES):
        acc += res.results[c]["out"]
    acc += b_proj
    return acc.reshape(B, T, C).astype(np.float32), res


def kernel(**inputs):
    out, _ = _run(inputs, trace=False)
    return out


def kernel_traced(**inputs):
    _, res = _run(inputs, trace=True)
    return res
